# revision 17
# baseline (speedup 1.0000x reference)
"""Trainium2 Bass kernel for the GSC Vanilla SNN problem.

3-layer LIF spiking net, S=101 timesteps, B=2048 batch, data-parallel over
batch across 8 NeuronCores (256 rows per core).

Math (per layer, per step, spikingjelly LIF with tau=2, v_th=1, hard reset),
state kept in "a-space" (a_t = v_{t-1} + c_t, v_t = a_t*(a_t<2)*0.5):
    a_t = select(a_{t-1} < 2, a_{t-1}*0.5, 0) + c_t
one fused custom DVE op (LIF_STEP_ANT) per layer-step, reading the matmul
current c_t DIRECTLY FROM PSUM (in1) -- there is no separate PSUM->SBUF
evacuation and no separate membrane tensor.

Spikes feed the next layer in SIGN convention y = sign(a - 2) in {-1,+1},
computed by the ACT engine (one [128,1024] Sign activation per layer-pair,
off the state critical path). With s = (1+y)/2:
    s @ W + b == y @ (W/2) + (b + colsum(W)/2)
so packed mid weights are W/2 with the bias' = b + colsum(W)/2 riding the
always -1 pad lanes h=200/201 as NEGATED hi/lo rows. Layer 1 biases ride
the two x ones-rows. Readout accumulates R = sum_t y3_t @ (Wr/2) in PSUM
(pair-interleaved [12, 512]); host applies base' + R/S, log_softmax.

Schedule: layer l is SKEWED by 2*l timesteps; timesteps processed in PAIRS
(N=512 matmuls). Per (layer, pair) one 2-bank PSUM tile [128, 2m, 2p, 256]
holds both m-chunks so the fused op and sign op span both chunks at once.

Engines: PE matmuls only; DVE runs the fused LIF op (6 per pair); ACT runs
the pair-wide Sign (3 per pair); Pool idle (GPSIMD measured ~8us per op --
unusable).
"""

import numpy as np
import ml_dtypes

S = 101
D = 120            # C*M input features
DA = 122           # augmented with 2 ones-rows for hi/lo bias
H = 200
HP = 256           # padded hidden
DOUT = 12
NCORES = 8
B_FULL = 2048
BC = B_FULL // NCORES   # 256 batch rows per core
TB = 8                  # x DMA block (timesteps per DMA)
NL = 3                  # layers
SKEW = 2                # timestep skew between adjacent layers

_bf16 = ml_dtypes.bfloat16

_BUILD_CACHE = {}


def _register_lif_op():
    """Additively register the fused LIF-step custom DVE op:
    out = select(in0 < s0, in0*s1, 0) + in1."""
    import concourse.dve_ops as dom
    from concourse.dve_spec import Spec, Src0, Src1, C0, C1, Zero, select, lower
    from concourse.dve_uop import DveOpSpec

    NAME = "LIF_STEP_ANT"
    if NAME in dom._SUB_OPCODE_FOR_NAME:
        return next(op for op in dom.OPS if op.name == NAME)
    body = select(Src0 < C0, Src0 * C1, Zero) + Src1
    spec = Spec(
        body=body,
        reference=lambda in0, in1, s0, s1, imm2: (
            np.where(in0.astype(np.float32) < s0, in0.astype(np.float32) * s1, 0.0)
            + in1
        ).astype(np.float32),
    )
    row = dom._CUSTOM_DVE_ROW_BASE + len(dom.OPS)
    shas = {}
    for ver in ("v3", "v4"):
        u = lower(spec, ver=ver)
        s = DveOpSpec(name=NAME, opcode=row, uops=u, rd1_en=True)
        shas[ver] = s.sha(ver)
    op = dom.DveOp(name=NAME, spec=spec, subdim=False, uops_sha=shas)
    dom._SUB_OPCODE_FOR_NAME[NAME] = row
    dom.OPS.append(op)
    dom.CUSTOM_DVE_SPECS[NAME] = spec
    return op


def _build(s_steps, bc, tb):
    """Build + compile the Bass program for one core. Returns nc."""
    import concourse.bacc as bacc
    import concourse.mybir as mybir
    import concourse.tile as tile

    lif_op = _register_lif_op()
    dt = mybir.dt
    alu = mybir.AluOpType
    P = 128
    B2 = 2 * bc
    n_rpairs = (s_steps + 1) // 2          # real pairs per layer (last may be single)
    n_mpairs = n_rpairs + NL - 1           # mega-pairs

    def pair_len(q):  # steps in real pair q
        return min(2, s_steps - 2 * q)

    nc = bacc.Bacc("TRN2", target_bir_lowering=False, debug=False)

    x_d = nc.dram_tensor("x", [DA, s_steps * bc], dt.bfloat16, kind="ExternalInput")
    w1_d = nc.dram_tensor("w1", [DA, HP], dt.bfloat16, kind="ExternalInput")
    w2_d = nc.dram_tensor("w2", [2, P, HP], dt.bfloat16, kind="ExternalInput")
    w3_d = nc.dram_tensor("w3", [2, P, HP], dt.bfloat16, kind="ExternalInput")
    wr_d = nc.dram_tensor("wr", [2, P, DOUT], dt.bfloat16, kind="ExternalInput")
    out_d = nc.dram_tensor("out", [DOUT, B2], dt.float32, kind="ExternalOutput")

    with tile.TileContext(nc) as tc:
        with (
            tc.tile_pool(name="const", bufs=1) as constp,
            tc.tile_pool(name="xp", bufs=6) as xp,
            tc.tile_pool(name="ap_", bufs=6) as ap_,
            tc.tile_pool(name="zp_", bufs=6) as zp_,
            tc.tile_pool(name="outp", bufs=1) as outp,
            tc.tile_pool(name="ps", bufs=1, space="PSUM") as psp,
            tc.tile_pool(name="psr", bufs=1, space="PSUM") as psrp,
        ):
            # DMA issue order = earliest-need order on the single sync queue
            # pair (each dma_start costs ~1us fixed): w1 and x-chunk0 feed
            # pair 0; w2 feeds J=2; wr feeds the readout at J=3; w3 feeds
            # J=4. w2/w3/wr ride ONE wide-tile DMA each ([P, 2k, cols]).
            w1 = constp.tile([DA, HP], dt.bfloat16)
            nc.sync.dma_start(w1[:], w1_d[:])
            xb0 = xp.tile([DA, tb * bc], dt.bfloat16, name="xb")
            nc.scalar.dma_start(xb0[:, 0 : 2 * bc], x_d[:, 0 : 2 * bc])
            w2w = constp.tile([P, 2, HP], dt.bfloat16)
            nc.sync.dma_start(w2w[:], w2_d.rearrange("k p h -> p k h"))
            nc.scalar.dma_start(xb0[:, 2 * bc : 4 * bc], x_d[:, 2 * bc : 4 * bc])
            wrw = constp.tile([P, 2, DOUT], dt.bfloat16)
            nc.sync.dma_start(wrw[:], wr_d.rearrange("k p h -> p k h"))
            nc.scalar.dma_start(xb0[:, 4 * bc : 6 * bc], x_d[:, 4 * bc : 6 * bc])
            nc.scalar.dma_start(xb0[:, 6 * bc : 8 * bc], x_d[:, 6 * bc : 8 * bc])
            w3w = constp.tile([P, 2, HP], dt.bfloat16)
            nc.sync.dma_start(w3w[:], w3_d.rearrange("k p h -> p k h"))
            # block 1 chunked: its first pairs are needed while warmup is
            # still DMA-bandwidth-bound
            xb1 = xp.tile([DA, tb * bc], dt.bfloat16, name="xb")
            for k in range(0, tb * bc, 2 * bc):
                nc.scalar.dma_start(
                    xb1[:, k : k + 2 * bc],
                    x_d[:, tb * bc + k : tb * bc + k + 2 * bc],
                )
            bias_t = constp.tile([P, 1], dt.float32)
            nc.vector.memset(bias_t[:], -2.0)
            zero_t = constp.tile([P, 2, bc], dt.bfloat16)
            nc.vector.memset(zero_t[:], 0.0)

            R = psrp.tile([DOUT, B2], dt.float32)

            w23 = [w2w, w3w]
            xbs = {0: xb0, 1: xb1}

            def ensure_xblock(bi, fine=False):
                # DMA x block bi ([DA, tb*bc]); fine=True streams it in
                # per-pair chunks so the first matmul unblocks early
                t0 = bi * tb
                if bi in xbs or t0 >= s_steps:
                    return
                ncols = min(tb, s_steps - t0) * bc
                xt = xp.tile([DA, tb * bc], dt.bfloat16, name="xb")
                if fine:
                    step2 = 2 * bc
                    for k in range(0, ncols, step2):
                        ke = min(k + step2, ncols)
                        nc.scalar.dma_start(
                            xt[:, k:ke], x_d[:, t0 * bc + k : t0 * bc + ke]
                        )
                else:
                    nc.scalar.dma_start(
                        xt[:, 0:ncols], x_d[:, t0 * bc : t0 * bc + ncols]
                    )
                xbs[bi] = xt

            zprev_tiles = [None, None]  # y supertiles of mega-pairs J-1, J
            aprev_tiles = [None]        # A supertile of mega-pair J-1

            for J in range(n_mpairs):
                # ---- matmul phase: layer l works real pair q = J - l ----
                lcs = [l for l in range(NL) if 0 <= J - l < n_rpairs]
                # per-pair state supertile in a-space: [128, l, p, m, b]
                A_cur = ap_.tile([P, NL, 2, 2, bc], dt.bfloat16, name="A")
                z_cur = zp_.tile([P, NL, 2, 2, bc], dt.bfloat16, name="z")
                # per-layer 2-bank PSUM tile: [128, m, p, b]
                pst = {}
                for l in lcs:
                    q = J - l
                    np_ = pair_len(q)
                    W = np_ * bc
                    if l == 0 and (2 * q) % tb == 0:
                        bi = (2 * q) // tb
                        ensure_xblock(bi, fine=(bi == 0))
                        ensure_xblock(bi + 1)  # prefetch 4 pairs ahead
                    cm = psp.tile([P, 2, 2, bc], dt.float32, name="cb", bufs=3)
                    pst[l] = cm
                    for m in range(2):
                        mout = cm[:, m, 0:np_, :].rearrange("p s b -> p (s b)")
                        if l == 0:
                            xb = xbs[(2 * q) // tb]
                            xpair = xb[:, (2 * q % tb) * bc : (2 * q % tb) * bc + W]
                            nc.tensor.matmul(
                                mout, w1[:, m * P : (m + 1) * P], xpair,
                                start=True, stop=True,
                                skip_group_check=True,
                            )
                        else:
                            zsrc = zprev_tiles[-1]
                            ww = w23[l - 1]
                            order = ((0, True), (1, False))
                            if m == 1:
                                order = ((1, True), (0, False))
                            for kk, first in order:
                                nc.tensor.matmul(
                                    mout,
                                    ww[:, kk, m * P : (m + 1) * P],
                                    zsrc[:, l - 1, kk, 0:np_, :],
                                    start=first, stop=not first,
                                    skip_group_check=True,
                                )

                # ---- readout: y3 of real pair q3 = (J-1) - 2 ----
                if zprev_tiles[-1] is not None:
                    q3 = (J - 1) - 2
                    if 0 <= q3 < n_rpairs:
                        np3 = pair_len(q3)
                        W3_ = np3 * bc
                        zsrc = zprev_tiles[-1]
                        nc.tensor.matmul(
                            R[:, 0:W3_], wrw[:, 0, :], zsrc[:, 2, 0, 0:np3, :],
                            start=(q3 == 0), stop=False, skip_group_check=True,
                        )
                        nc.tensor.matmul(
                            R[:, 0:W3_], wrw[:, 1, :], zsrc[:, 2, 1, 0:np3, :],
                            start=False, stop=(q3 == n_rpairs - 1),
                            skip_group_check=True,
                        )

                # ---- elementwise phase: fused LIF per (layer, step) on DVE,
                # pair-wide Sign per layer on ACT ----
                for p in range(2):
                    lr = [l for l in lcs if p < pair_len(J - l)]
                    for l in lr:
                        q = J - l
                        if p == 0:
                            ain = (
                                zero_t[:]
                                if q == 0
                                else aprev_tiles[-1][:, l, 1, :, :]
                            )
                        else:
                            ain = A_cur[:, l, 0, :, :]
                        nc.vector._custom_dve(
                            lif_op,
                            out=A_cur[:, l, p, :, :],
                            in0=ain,
                            in1=pst[l][:, :, p, :],
                            s0=2.0, s1=0.5,
                        )
                for l in lcs:
                    np_ = pair_len(J - l)
                    # y = sign(a - 2): out dims (m, p, b) <- in dims (m, p, b)
                    nc.scalar.activation(
                        z_cur[:, l, :, 0:np_, :],
                        A_cur[:, l, 0:np_, :, :].rearrange("p s m b -> p m s b"),
                        mybir.ActivationFunctionType.Sign,
                        bias=bias_t[:],
                    )

                zprev_tiles = [zprev_tiles[-1], z_cur]
                aprev_tiles = [A_cur]

            # ---- final readout for the last mega-pair's y3 ----
            q3 = n_mpairs - 3
            if 0 <= q3 < n_rpairs:
                np3 = pair_len(q3)
                W3_ = np3 * bc
                zsrc = zprev_tiles[-1]
                nc.tensor.matmul(
                    R[:, 0:W3_], wrw[:, 0, :], zsrc[:, 2, 0, 0:np3, :],
                    start=(q3 == 0), stop=False, skip_group_check=True,
                )
                nc.tensor.matmul(
                    R[:, 0:W3_], wrw[:, 1, :], zsrc[:, 2, 1, 0:np3, :],
                    start=False, stop=(q3 == n_rpairs - 1),
                    skip_group_check=True,
                )

            out_sb = outp.tile([DOUT, B2], dt.float32)
            nc.vector.tensor_copy(out_sb[:], R[:])
            nc.sync.dma_start(out_d[:], out_sb[:])

    nc.compile()
    return nc


def _get_nc(s_steps=S, bc=BC, tb=TB):
    key = (s_steps, bc, tb)
    if key not in _BUILD_CACHE:
        _BUILD_CACHE[key] = _build(s_steps, bc, tb)
    return _BUILD_CACHE[key]


def _hi_lo(v):
    hi = v.astype(_bf16)
    lo = (v - hi.astype(np.float64)).astype(_bf16)
    return hi, lo


def _prep_weights(W1, b1, W2, b2, W3, b3, Wr, br):
    """Host-side weight packing for the y = sign(a-2) in {-1,+1} convention.
    s = (1+y)/2, so s@W + b = y@(W/2) + (b + colsum(W)/2); pad spike lanes
    (a=0 -> y=-1) carry the NEGATED hi/lo of the folded bias."""
    P = 128

    w1p = np.zeros((DA, HP), np.float64)
    w1p[:D, :H] = np.asarray(W1, np.float64)
    w1p_bf = w1p.astype(_bf16)
    bh = np.zeros(HP, np.float64)
    bh[:H] = np.asarray(b1, np.float64)
    w1p_bf[D], w1p_bf[D + 1] = _hi_lo(bh)

    def mid(W, b):
        wp = np.zeros((HP, HP), np.float64)
        wp[:H, :H] = np.asarray(W, np.float64) * 0.5
        wpb = wp.astype(_bf16)
        bh = np.zeros(HP, np.float64)
        bh[:H] = (
            np.asarray(b, np.float64)
            + np.asarray(W, np.float64).sum(axis=0) * 0.5
        )
        hi, lo = _hi_lo(bh)
        wpb[H] = -hi.astype(np.float64)
        wpb[H + 1] = -lo.astype(np.float64)
        return wpb.reshape(2, P, HP)

    w2p = mid(W2, b2)
    w3p = mid(W3, b3)

    wrp = np.zeros((HP, DOUT), np.float64)
    wrp[:H] = np.asarray(Wr, np.float64) * 0.5
    wrp_bf = wrp.astype(_bf16)

    base = (
        np.asarray(br, np.float64)
        + np.asarray(Wr, np.float64).sum(axis=0) * 0.5
    ).astype(np.float32)
    return {"w1": w1p_bf, "w2": w2p, "w3": w3p, "wr": wrp_bf.reshape(2, P, DOUT)}, base


def _prep_x(x):
    """[B,C,S,M] f32 -> per-core [DA, S*bc] bf16 list (with two ones-rows)."""
    x = np.asarray(x, np.float32)
    B = x.shape[0]
    bc = B // NCORES
    xt = np.ascontiguousarray(x.transpose(1, 3, 2, 0)).reshape(D, S, B).astype(_bf16)
    outs = []
    for i in range(NCORES):
        xc = np.ones((DA, S * bc), dtype=_bf16)
        xc[:D] = xt[:, :, i * bc : (i + 1) * bc].reshape(D, S * bc)
        outs.append(xc)
    return outs


def _postprocess(R_list, base):
    """R per core [12, 2*bc] (pair-interleaved) -> full [B, 12] log_softmax."""
    outs = []
    for R in R_list:
        bc = R.shape[1] // 2
        Rs = (R[:, :bc] + R[:, bc:]).astype(np.float32)
        o = base[None, :] + Rs.T / np.float32(S)
        m = o.max(axis=1, keepdims=True)
        z = o - m
        lse = np.log(np.exp(z).sum(axis=1, keepdims=True))
        outs.append(z - lse)
    return np.concatenate(outs, axis=0).astype(np.float32)


def _ensure_ntff_hook():
    """Inject antenv.axon_hooks (NTFF profile hook) if the image lacks it."""
    import sys
    try:
        from antenv.axon_hooks import get_axon_ntff_profile_hook  # noqa: F401
        return True
    except ImportError:
        pass
    import contextlib
    import ctypes
    import types

    so_path = "/opt/axon/libaxon_pjrt.so"
    try:
        lib = ctypes.CDLL(so_path)
    except OSError:
        return False
    if not hasattr(lib, "axon_start_nrt_profile"):
        return False
    lib.axon_start_nrt_profile.argtypes = [
        ctypes.POINTER(ctypes.c_int64),
        ctypes.c_size_t,
    ]
    lib.axon_start_nrt_profile.restype = ctypes.c_int64
    lib.axon_stop_nrt_profile.argtypes = [ctypes.c_char_p]
    lib.axon_stop_nrt_profile.restype = ctypes.c_int64

    @contextlib.contextmanager
    def _hook(output_dir, device_ids):
        import jax

        jax.devices()
        if device_ids:
            ids = (ctypes.c_int64 * len(device_ids))(*device_ids)
            rc = lib.axon_start_nrt_profile(ids, len(device_ids))
        else:
            rc = lib.axon_start_nrt_profile(None, 0)
        if rc != 0:
            raise RuntimeError(f"axon_start_nrt_profile rc={rc}")
        try:
            yield
        finally:
            n = lib.axon_stop_nrt_profile(str(output_dir).encode())
            if n < 0:
                raise RuntimeError(f"axon_stop_nrt_profile rc={n}")

    mod = types.ModuleType("antenv.axon_hooks")
    mod._hook = _hook
    mod.get_axon_ntff_profile_hook = lambda: _hook
    mod.set_axon_ntff_profile_hook = lambda h: setattr(mod, "_hook", h)
    import antenv

    sys.modules["antenv.axon_hooks"] = mod
    antenv.axon_hooks = mod
    return True


def kernel(x, W1, b1, W2, b2, W3, b3, Wr, br, _trace=False):
    from concourse.bass_utils import run_bass_kernel_spmd

    if _trace:
        _trace = _ensure_ntff_hook()
    nc = _get_nc()
    wmap, base = _prep_weights(W1, b1, W2, b2, W3, b3, Wr, br)
    xs = _prep_x(x)
    in_maps = [{**wmap, "x": xs[i]} for i in range(NCORES)]
    res = run_bass_kernel_spmd(
        nc, in_maps, core_ids=list(range(NCORES)), trace=_trace
    )
    R_list = [res.results[i]["out"] for i in range(NCORES)]
    out = _postprocess(R_list, base)
    if _trace:
        kernel.last_exec_time_ns = res.exec_time_ns
        kernel.last_results = res
    return out


kernel.last_exec_time_ns = None
kernel.last_results = None


# revision 19
# speedup vs baseline: 1.1230x; 1.1230x over previous
"""Trainium2 Bass kernel for the GSC Vanilla SNN problem.

3-layer LIF spiking net, S=101 timesteps, B=2048 batch, data-parallel over
batch across 8 NeuronCores (256 rows per core).

Math (per layer, per step, spikingjelly LIF with tau=2, v_th=1, hard reset),
state kept in "a-space" (a_t = v_{t-1} + c_t, v_t = a_t*(a_t<2)*0.5):
    a_t = select(a_{t-1} < 2, a_{t-1}*0.5, 0) + c_t
one fused custom DVE op (LIF_STEP_ANT) per layer-step, reading the matmul
current c_t DIRECTLY FROM PSUM (in1) -- there is no separate PSUM->SBUF
evacuation and no separate membrane tensor.

Spikes feed the next layer in SIGN convention y = sign(a - 2) in {-1,+1},
computed by the ACT engine (one [128,1024] Sign activation per layer-pair,
off the state critical path). With s = (1+y)/2:
    s @ W + b == y @ (W/2) + (b + colsum(W)/2)
so packed mid weights are W/2 with the bias' = b + colsum(W)/2 riding the
always -1 pad lanes h=200/201 as NEGATED hi/lo rows. Layer 1 biases ride
the two x ones-rows. Readout accumulates R = sum_t y3_t @ (Wr/2) in PSUM
(pair-interleaved [12, 512]); host applies base' + R/S, log_softmax.

Schedule: layer l is SKEWED by 2*l timesteps; timesteps processed in PAIRS
(N=512 matmuls). Per (layer, pair) one 2-bank PSUM tile [128, 2m, 2p, 256]
holds both m-chunks so the fused op and sign op span both chunks at once.

Engines: PE matmuls only; DVE runs the fused LIF op (6 per pair); ACT runs
the pair-wide Sign (3 per pair); Pool idle (GPSIMD measured ~8us per op --
unusable).
"""

import numpy as np
import ml_dtypes

S = 101
D = 120            # C*M input features
DA = 122           # augmented with 2 ones-rows for hi/lo bias
H = 200
HP = 256           # padded hidden
DOUT = 12
NCORES = 8
B_FULL = 2048
BC = B_FULL // NCORES   # 256 batch rows per core
TB = 8                  # x DMA block (timesteps per DMA)
NL = 3                  # layers
SKEW = 2                # timestep skew between adjacent layers

_bf16 = ml_dtypes.bfloat16

_BUILD_CACHE = {}


def _register_lif_op():
    """Additively register the fused LIF-step custom DVE op:
    out = select(in0 < s0, in0*s1, 0) + in1."""
    import concourse.dve_ops as dom
    from concourse.dve_spec import Spec, Src0, Src1, C0, C1, Zero, select, lower
    from concourse.dve_uop import DveOpSpec

    NAME = "LIF_STEP_ANT"
    if NAME in dom._SUB_OPCODE_FOR_NAME:
        return next(op for op in dom.OPS if op.name == NAME)
    body = select(Src0 < C0, Src0 * C1, Zero) + Src1
    spec = Spec(
        body=body,
        reference=lambda in0, in1, s0, s1, imm2: (
            np.where(in0.astype(np.float32) < s0, in0.astype(np.float32) * s1, 0.0)
            + in1
        ).astype(np.float32),
    )
    row = dom._CUSTOM_DVE_ROW_BASE + len(dom.OPS)
    shas = {}
    for ver in ("v3", "v4"):
        u = lower(spec, ver=ver)
        s = DveOpSpec(name=NAME, opcode=row, uops=u, rd1_en=True)
        shas[ver] = s.sha(ver)
    op = dom.DveOp(name=NAME, spec=spec, subdim=False, uops_sha=shas)
    dom._SUB_OPCODE_FOR_NAME[NAME] = row
    dom.OPS.append(op)
    dom.CUSTOM_DVE_SPECS[NAME] = spec
    return op


def _build(s_steps, bc, tb):
    """Build + compile the Bass program for one core. Returns nc."""
    import concourse.bacc as bacc
    import concourse.mybir as mybir
    import concourse.tile as tile

    lif_op = _register_lif_op()
    dt = mybir.dt
    alu = mybir.AluOpType
    P = 128
    B2 = 2 * bc
    n_rpairs = (s_steps + 1) // 2          # real pairs per layer (last may be single)
    n_mpairs = n_rpairs + NL - 1           # mega-pairs

    def pair_len(q):  # steps in real pair q
        return min(2, s_steps - 2 * q)

    nc = bacc.Bacc("TRN2", target_bir_lowering=False, debug=False)

    x_d = nc.dram_tensor("x", [DA, s_steps * bc], dt.bfloat16, kind="ExternalInput")
    w1_d = nc.dram_tensor("w1", [DA, HP], dt.bfloat16, kind="ExternalInput")
    w2_d = nc.dram_tensor("w2", [2, P, HP], dt.bfloat16, kind="ExternalInput")
    w3_d = nc.dram_tensor("w3", [2, P, HP], dt.bfloat16, kind="ExternalInput")
    wr_d = nc.dram_tensor("wr", [2, P, DOUT], dt.bfloat16, kind="ExternalInput")
    out_d = nc.dram_tensor("out", [DOUT, B2], dt.float32, kind="ExternalOutput")

    with tile.TileContext(nc) as tc:
        with (
            tc.tile_pool(name="const", bufs=1) as constp,
            tc.tile_pool(name="xp", bufs=6) as xp,
            tc.tile_pool(name="ap_", bufs=6) as ap_,
            tc.tile_pool(name="zp_", bufs=6) as zp_,
            tc.tile_pool(name="outp", bufs=1) as outp,
            tc.tile_pool(name="ps", bufs=1, space="PSUM") as psp,
            tc.tile_pool(name="psr", bufs=1, space="PSUM") as psrp,
        ):
            # DMA issue order = earliest-need order on the single sync queue
            # pair (each dma_start costs ~1us fixed): w1 and x-chunk0 feed
            # pair 0; w2 feeds J=2; wr feeds the readout at J=3; w3 feeds
            # J=4. w2/w3/wr ride ONE wide-tile DMA each ([P, 2k, cols]).
            w1 = constp.tile([DA, HP], dt.bfloat16)
            nc.sync.dma_start(w1[:], w1_d[:])
            xb0 = xp.tile([DA, tb * bc], dt.bfloat16, name="xb")
            nc.sync.dma_start(xb0[:, 0:bc], x_d[:, 0:bc])
            nc.sync.dma_start(xb0[:, bc : 2 * bc], x_d[:, bc : 2 * bc])
            w2w = constp.tile([P, 2, HP], dt.bfloat16)
            nc.sync.dma_start(w2w[:], w2_d.rearrange("k p h -> p k h"))
            nc.sync.dma_start(xb0[:, 2 * bc : 4 * bc], x_d[:, 2 * bc : 4 * bc])
            wrw = constp.tile([P, 2, DOUT], dt.bfloat16)
            nc.sync.dma_start(wrw[:], wr_d.rearrange("k p h -> p k h"))
            nc.sync.dma_start(xb0[:, 4 * bc : 6 * bc], x_d[:, 4 * bc : 6 * bc])
            nc.sync.dma_start(xb0[:, 6 * bc : 8 * bc], x_d[:, 6 * bc : 8 * bc])
            w3w = constp.tile([P, 2, HP], dt.bfloat16)
            nc.sync.dma_start(w3w[:], w3_d.rearrange("k p h -> p k h"))
            # block 1 chunked: its first pairs are needed while warmup is
            # still DMA-bandwidth-bound
            xb1 = xp.tile([DA, tb * bc], dt.bfloat16, name="xb")
            for k in range(0, tb * bc, 2 * bc):
                nc.sync.dma_start(
                    xb1[:, k : k + 2 * bc],
                    x_d[:, tb * bc + k : tb * bc + k + 2 * bc],
                )
            bias_t = constp.tile([P, 1], dt.float32)
            nc.vector.memset(bias_t[:], -2.0)
            zero_t = constp.tile([P, 2, bc], dt.bfloat16)
            nc.vector.memset(zero_t[:], 0.0)

            R = psrp.tile([DOUT, B2], dt.float32)

            w23 = [w2w, w3w]
            xbs = {0: xb0, 1: xb1}

            def ensure_xblock(bi, fine=False):
                # DMA x block bi ([DA, tb*bc]); fine=True streams it in
                # per-pair chunks so the first matmul unblocks early
                t0 = bi * tb
                if bi in xbs or t0 >= s_steps:
                    return
                ncols = min(tb, s_steps - t0) * bc
                xt = xp.tile([DA, tb * bc], dt.bfloat16, name="xb")
                if fine:
                    step2 = 2 * bc
                    for k in range(0, ncols, step2):
                        ke = min(k + step2, ncols)
                        nc.sync.dma_start(
                            xt[:, k:ke], x_d[:, t0 * bc + k : t0 * bc + ke]
                        )
                else:
                    nc.sync.dma_start(
                        xt[:, 0:ncols], x_d[:, t0 * bc : t0 * bc + ncols]
                    )
                xbs[bi] = xt

            zprev_tiles = [None, None]  # y supertiles of mega-pairs J-1, J
            aprev_tiles = [None]        # A supertile of mega-pair J-1

            for J in range(n_mpairs):
                # ---- matmul phase: layer l works real pair q = J - l ----
                lcs = [l for l in range(NL) if 0 <= J - l < n_rpairs]
                # per-pair state supertile in a-space: [128, l, p, m, b]
                A_cur = ap_.tile([P, NL, 2, 2, bc], dt.bfloat16, name="A")
                z_cur = zp_.tile([P, NL, 2, 2, bc], dt.bfloat16, name="z")
                # per-layer 2-bank PSUM tile: [128, m, p, b]
                pst = {}
                for l in lcs:
                    q = J - l
                    np_ = pair_len(q)
                    W = np_ * bc
                    if l == 0 and (2 * q) % tb == 0:
                        bi = (2 * q) // tb
                        ensure_xblock(bi, fine=(bi == 0))
                        ensure_xblock(bi + 1)  # prefetch 4 pairs ahead
                    cm = psp.tile([P, 2, 2, bc], dt.float32, name="cb", bufs=3)
                    pst[l] = cm
                    for m in range(2):
                        mout = cm[:, m, 0:np_, :].rearrange("p s b -> p (s b)")
                        if l == 0:
                            xb = xbs[(2 * q) // tb]
                            if q == 0:
                                for pp in range(np_):
                                    nc.tensor.matmul(
                                        cm[:, m, pp : pp + 1, :].rearrange(
                                            "p s b -> p (s b)"
                                        ),
                                        w1[:, m * P : (m + 1) * P],
                                        xb[:, pp * bc : (pp + 1) * bc],
                                        start=True, stop=True,
                                        skip_group_check=True,
                                    )
                            else:
                                xpair = xb[:, (2 * q % tb) * bc : (2 * q % tb) * bc + W]
                                nc.tensor.matmul(
                                    mout, w1[:, m * P : (m + 1) * P], xpair,
                                    start=True, stop=True,
                                    skip_group_check=True,
                                )
                        else:
                            zsrc = zprev_tiles[-1]
                            ww = w23[l - 1]
                            order = ((0, True), (1, False))
                            if m == 1:
                                order = ((1, True), (0, False))
                            for kk, first in order:
                                nc.tensor.matmul(
                                    mout,
                                    ww[:, kk, m * P : (m + 1) * P],
                                    zsrc[:, l - 1, kk, 0:np_, :],
                                    start=first, stop=not first,
                                    skip_group_check=True,
                                )

                # ---- readout: y3 of real pair q3 = (J-1) - 2 ----
                if zprev_tiles[-1] is not None:
                    q3 = (J - 1) - 2
                    if 0 <= q3 < n_rpairs:
                        np3 = pair_len(q3)
                        W3_ = np3 * bc
                        zsrc = zprev_tiles[-1]
                        nc.tensor.matmul(
                            R[:, 0:W3_], wrw[:, 0, :], zsrc[:, 2, 0, 0:np3, :],
                            start=(q3 == 0), stop=False, skip_group_check=True,
                        )
                        nc.tensor.matmul(
                            R[:, 0:W3_], wrw[:, 1, :], zsrc[:, 2, 1, 0:np3, :],
                            start=False, stop=(q3 == n_rpairs - 1),
                            skip_group_check=True,
                        )

                # ---- elementwise phase: fused LIF per (layer, step) on DVE,
                # pair-wide Sign per layer on ACT ----
                for p in range(2):
                    lr = [l for l in lcs if p < pair_len(J - l)]
                    for l in lr:
                        q = J - l
                        if p == 0:
                            ain = (
                                zero_t[:]
                                if q == 0
                                else aprev_tiles[-1][:, l, 1, :, :]
                            )
                        else:
                            ain = A_cur[:, l, 0, :, :]
                        nc.vector._custom_dve(
                            lif_op,
                            out=A_cur[:, l, p, :, :],
                            in0=ain,
                            in1=pst[l][:, :, p, :],
                            s0=2.0, s1=0.5,
                        )
                for l in lcs:
                    np_ = pair_len(J - l)
                    # y = sign(a - 2): out dims (m, p, b) <- in dims (m, p, b)
                    nc.scalar.activation(
                        z_cur[:, l, :, 0:np_, :],
                        A_cur[:, l, 0:np_, :, :].rearrange("p s m b -> p m s b"),
                        mybir.ActivationFunctionType.Sign,
                        bias=bias_t[:],
                    )

                zprev_tiles = [zprev_tiles[-1], z_cur]
                aprev_tiles = [A_cur]

            # ---- final readout for the last mega-pair's y3 ----
            q3 = n_mpairs - 3
            if 0 <= q3 < n_rpairs:
                np3 = pair_len(q3)
                W3_ = np3 * bc
                zsrc = zprev_tiles[-1]
                nc.tensor.matmul(
                    R[:, 0:W3_], wrw[:, 0, :], zsrc[:, 2, 0, 0:np3, :],
                    start=(q3 == 0), stop=False, skip_group_check=True,
                )
                nc.tensor.matmul(
                    R[:, 0:W3_], wrw[:, 1, :], zsrc[:, 2, 1, 0:np3, :],
                    start=False, stop=(q3 == n_rpairs - 1),
                    skip_group_check=True,
                )

            out_sb = outp.tile([DOUT, B2], dt.float32)
            nc.vector.tensor_copy(out_sb[:], R[:])
            nc.sync.dma_start(out_d[:], out_sb[:])

    nc.compile()
    return nc


def _get_nc(s_steps=S, bc=BC, tb=TB):
    key = (s_steps, bc, tb)
    if key not in _BUILD_CACHE:
        _BUILD_CACHE[key] = _build(s_steps, bc, tb)
    return _BUILD_CACHE[key]


def _hi_lo(v):
    hi = v.astype(_bf16)
    lo = (v - hi.astype(np.float64)).astype(_bf16)
    return hi, lo


def _prep_weights(W1, b1, W2, b2, W3, b3, Wr, br):
    """Host-side weight packing for the y = sign(a-2) in {-1,+1} convention.
    s = (1+y)/2, so s@W + b = y@(W/2) + (b + colsum(W)/2); pad spike lanes
    (a=0 -> y=-1) carry the NEGATED hi/lo of the folded bias."""
    P = 128

    w1p = np.zeros((DA, HP), np.float64)
    w1p[:D, :H] = np.asarray(W1, np.float64)
    w1p_bf = w1p.astype(_bf16)
    bh = np.zeros(HP, np.float64)
    bh[:H] = np.asarray(b1, np.float64)
    w1p_bf[D], w1p_bf[D + 1] = _hi_lo(bh)

    def mid(W, b):
        wp = np.zeros((HP, HP), np.float64)
        wp[:H, :H] = np.asarray(W, np.float64) * 0.5
        wpb = wp.astype(_bf16)
        bh = np.zeros(HP, np.float64)
        bh[:H] = (
            np.asarray(b, np.float64)
            + np.asarray(W, np.float64).sum(axis=0) * 0.5
        )
        hi, lo = _hi_lo(bh)
        wpb[H] = -hi.astype(np.float64)
        wpb[H + 1] = -lo.astype(np.float64)
        return wpb.reshape(2, P, HP)

    w2p = mid(W2, b2)
    w3p = mid(W3, b3)

    wrp = np.zeros((HP, DOUT), np.float64)
    wrp[:H] = np.asarray(Wr, np.float64) * 0.5
    wrp_bf = wrp.astype(_bf16)

    base = (
        np.asarray(br, np.float64)
        + np.asarray(Wr, np.float64).sum(axis=0) * 0.5
    ).astype(np.float32)
    return {"w1": w1p_bf, "w2": w2p, "w3": w3p, "wr": wrp_bf.reshape(2, P, DOUT)}, base


def _prep_x(x):
    """[B,C,S,M] f32 -> per-core [DA, S*bc] bf16 list (with two ones-rows)."""
    x = np.asarray(x, np.float32)
    B = x.shape[0]
    bc = B // NCORES
    xt = np.ascontiguousarray(x.transpose(1, 3, 2, 0)).reshape(D, S, B).astype(_bf16)
    outs = []
    for i in range(NCORES):
        xc = np.ones((DA, S * bc), dtype=_bf16)
        xc[:D] = xt[:, :, i * bc : (i + 1) * bc].reshape(D, S * bc)
        outs.append(xc)
    return outs


def _postprocess(R_list, base):
    """R per core [12, 2*bc] (pair-interleaved) -> full [B, 12] log_softmax."""
    outs = []
    for R in R_list:
        bc = R.shape[1] // 2
        Rs = (R[:, :bc] + R[:, bc:]).astype(np.float32)
        o = base[None, :] + Rs.T / np.float32(S)
        m = o.max(axis=1, keepdims=True)
        z = o - m
        lse = np.log(np.exp(z).sum(axis=1, keepdims=True))
        outs.append(z - lse)
    return np.concatenate(outs, axis=0).astype(np.float32)


def _ensure_ntff_hook():
    """Inject antenv.axon_hooks (NTFF profile hook) if the image lacks it."""
    import sys
    try:
        from antenv.axon_hooks import get_axon_ntff_profile_hook  # noqa: F401
        return True
    except ImportError:
        pass
    import contextlib
    import ctypes
    import types

    so_path = "/opt/axon/libaxon_pjrt.so"
    try:
        lib = ctypes.CDLL(so_path)
    except OSError:
        return False
    if not hasattr(lib, "axon_start_nrt_profile"):
        return False
    lib.axon_start_nrt_profile.argtypes = [
        ctypes.POINTER(ctypes.c_int64),
        ctypes.c_size_t,
    ]
    lib.axon_start_nrt_profile.restype = ctypes.c_int64
    lib.axon_stop_nrt_profile.argtypes = [ctypes.c_char_p]
    lib.axon_stop_nrt_profile.restype = ctypes.c_int64

    @contextlib.contextmanager
    def _hook(output_dir, device_ids):
        import jax

        jax.devices()
        if device_ids:
            ids = (ctypes.c_int64 * len(device_ids))(*device_ids)
            rc = lib.axon_start_nrt_profile(ids, len(device_ids))
        else:
            rc = lib.axon_start_nrt_profile(None, 0)
        if rc != 0:
            raise RuntimeError(f"axon_start_nrt_profile rc={rc}")
        try:
            yield
        finally:
            n = lib.axon_stop_nrt_profile(str(output_dir).encode())
            if n < 0:
                raise RuntimeError(f"axon_stop_nrt_profile rc={n}")

    mod = types.ModuleType("antenv.axon_hooks")
    mod._hook = _hook
    mod.get_axon_ntff_profile_hook = lambda: _hook
    mod.set_axon_ntff_profile_hook = lambda h: setattr(mod, "_hook", h)
    import antenv

    sys.modules["antenv.axon_hooks"] = mod
    antenv.axon_hooks = mod
    return True


def kernel(x, W1, b1, W2, b2, W3, b3, Wr, br, _trace=False):
    from concourse.bass_utils import run_bass_kernel_spmd

    if _trace:
        _trace = _ensure_ntff_hook()
    nc = _get_nc()
    wmap, base = _prep_weights(W1, b1, W2, b2, W3, b3, Wr, br)
    xs = _prep_x(x)
    in_maps = [{**wmap, "x": xs[i]} for i in range(NCORES)]
    res = run_bass_kernel_spmd(
        nc, in_maps, core_ids=list(range(NCORES)), trace=_trace
    )
    R_list = [res.results[i]["out"] for i in range(NCORES)]
    out = _postprocess(R_list, base)
    if _trace:
        kernel.last_exec_time_ns = res.exec_time_ns
        kernel.last_results = res
    return out


kernel.last_exec_time_ns = None
kernel.last_results = None
